# revision 25
# baseline (speedup 1.0000x reference)
"""Multi-head attention (16 heads, D=1024, Lq=Lk=2048) on 8 TRN2 NeuronCores.

Sharding: tensor-parallel over heads. Core c owns heads {2c, 2c+1} =
feature slice S_c = [128c, 128c+128) of the QKV projection output and the
matching 128 columns of W_o's input dim. Each core computes, for each of its
two heads, an UNNORMALIZED partial of the output projection plus the softmax
denominators; the host divides by the denominators and all-reduces (sums)
over cores and heads.

Per-core device pipeline (matmuls in bf16, fp32 PSUM accumulation):
  qT/kT/vT = W*_c @ x.T             [128 feat, 2048 L]  (feat on partitions)
  v_nat    = transpose(vT) + ones column  (per head: [k, 64+1])
  per (q-quarter, k-block):
     scoresT = kT_h-slice.T @ qT_h  [128 k, 2x512 q]  (heads row-tiled at
               row groups 0/64 -> concurrent on the PE array)
     attnT   = exp(scoresT/8)       (one ScalarE instr; no max-subtraction:
               scores ~ N(0,1), exp stays in fp32/bf16 range)
     ctxT_h += v_nat_h.T @ attnT_h  [65, 512]  (row 64 = denominators)
  outA/outB = WoT_h.T @ ctx_h       per head, K=64 row-tiled pair
Host: out = (sum_c outA/s0 + outB/s1).T
"""

import sys

sys.path.insert(0, "/opt/trn_rl_repo")

from contextlib import ExitStack

import ml_dtypes
import numpy as np

import concourse.bass as bass
import concourse.tile as tile
from concourse import bacc, mybir
from concourse.bass_utils import run_bass_kernel_spmd
from concourse.masks import make_identity

F32 = mybir.dt.float32
F32R = mybir.dt.float32r
BF16 = mybir.dt.bfloat16
BF16NP = ml_dtypes.bfloat16

D = 1024
L = 2048  # Lq == Lk
H = 16
HD = 64
NCORES = 8
FPC = D // NCORES  # features per core = 128 (2 heads)
DK = D // 128  # 8 contraction chunks for projections
KB = L // 128  # 16 k-blocks
NQ = 4  # q quarters
QW = L // NQ  # 512


def build_kernel():
    nc = bacc.Bacc("TRN2", target_bir_lowering=False, debug=False)

    xT_d = nc.dram_tensor("xT", [D, L], BF16, kind="ExternalInput")
    hT_d = nc.dram_tensor("hT", [D, L], BF16, kind="ExternalInput")
    wqT_d = nc.dram_tensor("wqT", [D, FPC], BF16, kind="ExternalInput")
    wkT_d = nc.dram_tensor("wkT", [D, FPC], BF16, kind="ExternalInput")
    wvT_d = nc.dram_tensor("wvT", [D, FPC], BF16, kind="ExternalInput")
    woT_d = nc.dram_tensor("woT", [FPC, D], BF16, kind="ExternalInput")
    outAB_d = nc.dram_tensor("outAB", [2, D, L], BF16, kind="ExternalOutput")
    sums_d = nc.dram_tensor("sums", [2, L], F32, kind="ExternalOutput")

    with tile.TileContext(nc) as tc:
        _build_body(nc, tc, xT_d, hT_d, wqT_d, wkT_d, wvT_d, woT_d,
                    outAB_d, sums_d)

    nc.compile()
    return nc


def _build_body(nc, tc, xT_d, hT_d, wqT_d, wkT_d, wvT_d, woT_d,
                outAB_d, sums_d):
    ctx = ExitStack()
    with ctx:
        consts = ctx.enter_context(tc.tile_pool(name="consts", bufs=1))
        big = ctx.enter_context(tc.tile_pool(name="big", bufs=1))
        attn_p = ctx.enter_context(tc.tile_pool(name="attn", bufs=20))
        small = ctx.enter_context(tc.tile_pool(name="small", bufs=2))
        outp = ctx.enter_context(tc.tile_pool(name="outp", bufs=3))

        # ---- constants ----
        ident = consts.tile([128, 128], BF16)
        make_identity(nc, ident[:])
        ones_col = consts.tile([128, 1], BF16)
        nc.gpsimd.memset(ones_col[:], 1.0)

        wq_sb = consts.tile([128, DK, FPC], BF16)
        wk_sb = consts.tile([128, DK, FPC], BF16)
        wv_sb = consts.tile([128, DK, FPC], BF16)
        wo_sb = consts.tile([128, D], BF16)
        h_sb = big.tile([128, DK, L], BF16)
        x_sb = big.tile([128, DK, L], BF16)

        # critical path first: whole small weights in single triggers, then h
        # and x chunks interleaved so both k- and q-projections start early
        nc.sync.dma_start(wk_sb[:], wkT_d.ap().rearrange("(dk p) f -> p dk f", p=128))
        nc.sync.dma_start(wq_sb[:], wqT_d.ap().rearrange("(dk p) f -> p dk f", p=128))
        for dk in range(DK):
            nc.sync.dma_start(h_sb[:, dk, :], hT_d.ap()[dk * 128 : (dk + 1) * 128, :])
            nc.sync.dma_start(x_sb[:, dk, :], xT_d.ap()[dk * 128 : (dk + 1) * 128, :])
        nc.gpsimd.dma_start(wv_sb[:], wvT_d.ap().rearrange("(dk p) f -> p dk f", p=128))
        nc.gpsimd.dma_start(wo_sb[:], woT_d.ap()[:])

        qT = big.tile([128, L], BF16)
        kT = big.tile([128, L], BF16)
        vT = big.tile([128, L], BF16)
        # v natural per head, 16 blocks of [128, 65]; col 64 = ones
        v0 = big.tile([128, KB * 65], BF16)
        v1 = big.tile([128, KB * 65], BF16)
        for kb in range(KB):
            nc.vector.tensor_copy(v0[:, kb * 65 + 64 : kb * 65 + 65], ones_col[:])
            nc.vector.tensor_copy(v1[:, kb * 65 + 64 : kb * 65 + 65], ones_col[:])
        # unnormalized context, both heads packed [feat, q]
        ctxu = big.tile([128, L], BF16)

        def project(pool, dst, w_sb, src):
            # dk-outer: chunk-streaming friendly, needs 4 live psum tiles
            ps_tiles = [
                pool.tile([128, 512], F32, tag=f"ps_{nb}", name=f"ps_{nb}")
                for nb in range(4)
            ]
            for dk in range(DK):
                for nb in range(4):
                    nc.tensor.matmul(
                        ps_tiles[nb][:],
                        w_sb[:, dk, :],
                        src[:, dk, nb * 512 : (nb + 1) * 512],
                        start=(dk == 0),
                        stop=(dk == DK - 1),
                    )
            for nb in range(4):
                nc.vector.tensor_copy(dst[:, nb * 512 : (nb + 1) * 512], ps_tiles[nb][:])

        def project_compact(pool, dst, w_sb, src):
            # nb-outer: one live psum tile; used when the main pools are open
            for nb in range(4):
                ps = pool.tile([128, 512], F32, tag="psv", name="psv")
                for dk in range(DK):
                    nc.tensor.matmul(
                        ps[:],
                        w_sb[:, dk, :],
                        src[:, dk, nb * 512 : (nb + 1) * 512],
                        start=(dk == 0),
                        stop=(dk == DK - 1),
                    )
                nc.vector.tensor_copy(dst[:, nb * 512 : (nb + 1) * 512], ps[:])

        # k and q projections first so the attention loop can begin while the
        # v projection and transposes still stream through the PE
        with tc.tile_pool(name="proj_kq", bufs=2, space="PSUM") as proj_kq:
            project(proj_kq, kT, wk_sb, h_sb)
            project(proj_kq, qT, wq_sb, x_sb)

        # ---- main attention loop ----
        sc_pool = ctx.enter_context(tc.tile_pool(name="sc_ps", bufs=2, space="PSUM"))
        ctx_pool = ctx.enter_context(tc.tile_pool(name="ctx_ps", bufs=1, space="PSUM"))

        cps_all = {}

        def scores_exp(qq, kb):
            q0 = qq * QW
            k0 = kb * 128
            sc = sc_pool.tile([128, 2, QW], F32, tag="sc", name="sc")
            for h in range(2):
                f0 = h * 64
                nc.tensor.matmul(
                    sc[:, h, :],
                    kT[f0 : f0 + 64, k0 : k0 + 128],
                    qT[f0 : f0 + 64, q0 : q0 + QW],
                    start=True,
                    stop=True,
                    tile_position=(f0, 0),
                )
            at = attn_p.tile([128, 2, QW], BF16, tag="at", name="at")
            nc.scalar.activation(
                at[:], sc[:], mybir.ActivationFunctionType.Exp, scale=0.125
            )
            return at

        def ctx_mm(qq, kb, at):
            cps = cps_all[qq]
            for h, v_nat in ((0, v0), (1, v1)):
                nc.tensor.matmul(
                    cps[h][:],
                    v_nat[:, kb * 65 : (kb + 1) * 65],
                    at[:, h, :],
                    start=(kb == 0),
                    stop=(kb == KB - 1),
                )

        def drain_ctx(qq):
            # psum -> SBUF (bf16) + denominators out
            q0 = qq * QW
            cps = cps_all[qq]
            s_sb = small.tile([64, QW], F32, tag="s", name="s_sb")
            for h in range(2):
                nc.vector.tensor_copy(ctxu[h * 64 : (h + 1) * 64, q0 : q0 + QW], cps[h][0:64, :])
                nc.vector.tensor_copy(s_sb[32 * h : 32 * h + 1, :], cps[h][64:65, :])
            for h in range(2):
                nc.sync.dma_start(
                    sums_d.ap()[h : h + 1, q0 : q0 + QW], s_sb[32 * h : 32 * h + 1, :]
                )

        # quarter 0: scores/exp hoisted ahead of the v projection; its ctx
        # matmuls run after the transposes produce v_nat
        cps_all[0] = [
            ctx_pool.tile([65, QW], F32, tag=f"ctx{h}", name=f"ctx{h}") for h in range(2)
        ]
        at_q0 = [scores_exp(0, kb) for kb in range(KB)]

        with tc.tile_pool(name="proj_v", bufs=2, space="PSUM") as proj_v:
            project_compact(proj_v, vT, wv_sb, h_sb)
        with tc.tile_pool(name="tr_ps", bufs=2, space="PSUM") as tr_ps_pool:
            for kb in range(KB):
                trp = tr_ps_pool.tile([128, 128], BF16, name="trp")
                nc.tensor.transpose(trp[:], vT[:, kb * 128 : (kb + 1) * 128], ident[:])
                nc.vector.tensor_copy(v0[:, kb * 65 : kb * 65 + 64], trp[:, 0:64])
                nc.vector.tensor_copy(v1[:, kb * 65 : kb * 65 + 64], trp[:, 64:128])

        tail_pool = ctx.enter_context(tc.tile_pool(name="tail_ps", bufs=1, space="PSUM"))

        def tail_op(qq, m, last=False):
            # output projection chunk m for quarter qq: both heads as a
            # concurrent K=64 row-tiled pair, separate outputs. For the final
            # quarter the score slots are free: rotate through them too, and
            # split the drains between VectorE and the now-idle ScalarE.
            q0 = qq * QW
            if last and m % 3 != 0:
                op = sc_pool.tile([128, 2, QW], F32, tag="sc", name="op")
            else:
                op = tail_pool.tile([128, 2, QW], F32, tag="tail", name="op")
            for h in range(2):
                f0 = h * 64
                nc.tensor.matmul(
                    op[:, h, :],
                    wo_sb[f0 : f0 + 64, m * 128 : (m + 1) * 128],
                    ctxu[f0 : f0 + 64, q0 : q0 + QW],
                    start=True,
                    stop=True,
                    tile_position=(f0, 0),
                )
            o_sb = outp.tile([128, 2, QW], BF16, tag="o", name="o_sb")
            if last and m % 2 == 1:
                nc.scalar.copy(o_sb[:], op[:])
            else:
                nc.vector.tensor_copy(o_sb[:], op[:])
            # one trigger for both heads: [128, 2, QW] -> outAB[h, m-rows, q]
            nc.sync.dma_start(
                outAB_d.ap()[:, m * 128 : (m + 1) * 128, q0 : q0 + QW].rearrange(
                    "ab p q -> p ab q"
                ),
                o_sb[:],
            )

        at_q1 = []
        for kb in range(KB):
            ctx_mm(0, kb, at_q0[kb])
            if kb < 6:
                at_q1.append(scores_exp(1, kb))
        drain_ctx(0)

        for qq in range(1, NQ):
            cps_all[qq] = [
                ctx_pool.tile([65, QW], F32, tag=f"ctx{h}", name=f"ctx{h}")
                for h in range(2)
            ]
            for kb in range(KB):
                if qq == 1 and kb < 6:
                    at = at_q1[kb]
                else:
                    at = scores_exp(qq, kb)
                ctx_mm(qq, kb, at)
                if 3 <= kb < 11:
                    tail_op(qq - 1, kb - 3)
            drain_ctx(qq)
        for m in range(DK):
            tail_op(NQ - 1, m, last=True)


_NC_CACHE = None


def _get_nc():
    global _NC_CACHE
    if _NC_CACHE is None:
        _NC_CACHE = build_kernel()
    return _NC_CACHE


def make_in_maps(input_embeddings, history_hidden, W_q, W_k, W_v, W_o):
    xT = np.ascontiguousarray(np.asarray(input_embeddings, dtype=np.float32).T).astype(BF16NP)
    hT = np.ascontiguousarray(np.asarray(history_hidden, dtype=np.float32).T).astype(BF16NP)
    W_q = np.asarray(W_q, dtype=np.float32)
    W_k = np.asarray(W_k, dtype=np.float32)
    W_v = np.asarray(W_v, dtype=np.float32)
    W_o = np.asarray(W_o, dtype=np.float32)
    in_maps = []
    for c in range(NCORES):
        s = slice(c * FPC, (c + 1) * FPC)
        in_maps.append(
            {
                "xT": xT,
                "hT": hT,
                "wqT": np.ascontiguousarray(W_q[s, :].T).astype(BF16NP),
                "wkT": np.ascontiguousarray(W_k[s, :].T).astype(BF16NP),
                "wvT": np.ascontiguousarray(W_v[s, :].T).astype(BF16NP),
                "woT": np.ascontiguousarray(W_o[:, s].T).astype(BF16NP),
            }
        )
    return in_maps


def kernel(input_embeddings, history_hidden, W_q, W_k, W_v, W_o, _trace=False, _trace_kwargs=None):
    nc = _get_nc()
    in_maps = make_in_maps(input_embeddings, history_hidden, W_q, W_k, W_v, W_o)
    res = run_bass_kernel_spmd(
        nc, in_maps, core_ids=list(range(NCORES)), trace=_trace, **(_trace_kwargs or {})
    )
    acc = None
    for r in res.results:
        s = r["sums"].astype(np.float64)  # [2, L]
        ab = r["outAB"].astype(np.float64)  # [2, D, L]
        part = ab[0] / s[0][None, :] + ab[1] / s[1][None, :]
        acc = part if acc is None else acc + part
    out = np.ascontiguousarray(acc.T).astype(np.float32)
    if _trace:
        kernel._last_results = res
    return out


if __name__ == "__main__":
    rng = np.random.default_rng(0)
    ins = {
        "input_embeddings": rng.standard_normal((L, D), dtype=np.float32),
        "history_hidden": rng.standard_normal((L, D), dtype=np.float32),
        "W_q": rng.standard_normal((D, D), dtype=np.float32) / 32,
        "W_k": rng.standard_normal((D, D), dtype=np.float32) / 32,
        "W_v": rng.standard_normal((D, D), dtype=np.float32) / 32,
        "W_o": rng.standard_normal((D, D), dtype=np.float32) / 32,
    }
    out = kernel(**ins)
    print("kernel output", out.shape, out.dtype, float(np.abs(out).mean()))


# revision 27
# speedup vs baseline: 1.0638x; 1.0638x over previous
"""Multi-head attention (16 heads, D=1024, Lq=Lk=2048) on 8 TRN2 NeuronCores.

Sharding: tensor-parallel over heads. Core c owns heads {2c, 2c+1} =
feature slice S_c = [128c, 128c+128) of the QKV projection output and the
matching 128 columns of W_o's input dim. Each core computes, for each of its
two heads, an UNNORMALIZED partial of the output projection plus the softmax
denominators; the host divides by the denominators and all-reduces (sums)
over cores and heads.

Per-core device pipeline (matmuls in bf16, fp32 PSUM accumulation):
  qT/kT/vT = W*_c @ x.T             [128 feat, 2048 L]  (feat on partitions)
  v_nat    = transpose(vT) + ones column  (per head: [k, 64+1])
  per (q-quarter, k-block):
     scoresT = kT_h-slice.T @ qT_h  [128 k, 2x512 q]  (heads row-tiled at
               row groups 0/64 -> concurrent on the PE array)
     attnT   = exp(scoresT/8)       (one ScalarE instr; no max-subtraction:
               scores ~ N(0,1), exp stays in fp32/bf16 range)
     ctxT_h += v_nat_h.T @ attnT_h  [65, 512]  (row 64 = denominators)
  outA/outB = WoT_h.T @ ctx_h       per head, K=64 row-tiled pair
Host: out = (sum_c outA/s0 + outB/s1).T
"""

import sys

sys.path.insert(0, "/opt/trn_rl_repo")

from contextlib import ExitStack

import ml_dtypes
import numpy as np

import concourse.bass as bass
import concourse.tile as tile
from concourse import bacc, mybir
from concourse.bass_utils import run_bass_kernel_spmd
from concourse.masks import make_identity

F32 = mybir.dt.float32
F32R = mybir.dt.float32r
BF16 = mybir.dt.bfloat16
BF16NP = ml_dtypes.bfloat16

D = 1024
L = 2048  # Lq == Lk
H = 16
HD = 64
NCORES = 8
FPC = D // NCORES  # features per core = 128 (2 heads)
DK = D // 128  # 8 contraction chunks for projections
KB = L // 128  # 16 k-blocks
NQ = 4  # q quarters
QW = L // NQ  # 512


def build_kernel():
    nc = bacc.Bacc("TRN2", target_bir_lowering=False, debug=False)

    xT_d = nc.dram_tensor("xT", [D, L], BF16, kind="ExternalInput")
    hT_d = nc.dram_tensor("hT", [D, L], BF16, kind="ExternalInput")
    wqT_d = nc.dram_tensor("wqT", [D, FPC], BF16, kind="ExternalInput")
    wkT_d = nc.dram_tensor("wkT", [D, FPC], BF16, kind="ExternalInput")
    wvT_d = nc.dram_tensor("wvT", [D, FPC], BF16, kind="ExternalInput")
    woT_d = nc.dram_tensor("woT", [FPC, D], BF16, kind="ExternalInput")
    outAB_d = nc.dram_tensor("outAB", [2, D, L], BF16, kind="ExternalOutput")
    sums_d = nc.dram_tensor("sums", [2, L], F32, kind="ExternalOutput")

    with tile.TileContext(nc) as tc:
        _build_body(nc, tc, xT_d, hT_d, wqT_d, wkT_d, wvT_d, woT_d,
                    outAB_d, sums_d)

    nc.compile()
    return nc


def _build_body(nc, tc, xT_d, hT_d, wqT_d, wkT_d, wvT_d, woT_d,
                outAB_d, sums_d):
    ctx = ExitStack()
    with ctx:
        consts = ctx.enter_context(tc.tile_pool(name="consts", bufs=1))
        big = ctx.enter_context(tc.tile_pool(name="big", bufs=1))
        attn_p = ctx.enter_context(tc.tile_pool(name="attn", bufs=18))
        small = ctx.enter_context(tc.tile_pool(name="small", bufs=2))
        outp = ctx.enter_context(tc.tile_pool(name="outp", bufs=4))

        # ---- constants ----
        ident = consts.tile([128, 128], BF16)
        make_identity(nc, ident[:])
        ones_col = consts.tile([128, 1], BF16)
        nc.gpsimd.memset(ones_col[:], 1.0)

        wq_sb = consts.tile([128, DK, FPC], BF16)
        wk_sb = consts.tile([128, DK, FPC], BF16)
        wv_sb = consts.tile([128, DK, FPC], BF16)
        wo_sb = consts.tile([128, D], BF16)
        h_sb = big.tile([128, DK, L], BF16)
        x_sb = big.tile([128, DK, L], BF16)

        # critical path first: whole small weights in single triggers, then h
        # and x chunks interleaved so both k- and q-projections start early
        nc.sync.dma_start(wk_sb[:], wkT_d.ap().rearrange("(dk p) f -> p dk f", p=128))
        nc.sync.dma_start(wq_sb[:], wqT_d.ap().rearrange("(dk p) f -> p dk f", p=128))
        for dk in range(DK):
            nc.sync.dma_start(h_sb[:, dk, :], hT_d.ap()[dk * 128 : (dk + 1) * 128, :])
            nc.sync.dma_start(x_sb[:, dk, :], xT_d.ap()[dk * 128 : (dk + 1) * 128, :])
        nc.gpsimd.dma_start(wv_sb[:], wvT_d.ap().rearrange("(dk p) f -> p dk f", p=128))
        nc.gpsimd.dma_start(wo_sb[:], woT_d.ap()[:])

        qT = big.tile([128, L], BF16)
        kT = big.tile([128, L], BF16)
        vT = big.tile([128, L], BF16)
        # v natural per head, 16 blocks of [128, 65]; col 64 = ones
        v0 = big.tile([128, KB * 65], BF16)
        v1 = big.tile([128, KB * 65], BF16)
        for kb in range(KB):
            nc.vector.tensor_copy(v0[:, kb * 65 + 64 : kb * 65 + 65], ones_col[:])
            nc.vector.tensor_copy(v1[:, kb * 65 + 64 : kb * 65 + 65], ones_col[:])
        # unnormalized context, both heads packed [feat, q]
        ctxu = big.tile([128, L], BF16)

        def project(pool, dst, w_sb, src):
            # dk-outer: chunk-streaming friendly, needs 4 live psum tiles
            ps_tiles = [
                pool.tile([128, 512], F32, tag=f"ps_{nb}", name=f"ps_{nb}")
                for nb in range(4)
            ]
            for dk in range(DK):
                for nb in range(4):
                    nc.tensor.matmul(
                        ps_tiles[nb][:],
                        w_sb[:, dk, :],
                        src[:, dk, nb * 512 : (nb + 1) * 512],
                        start=(dk == 0),
                        stop=(dk == DK - 1),
                    )
            for nb in range(4):
                nc.vector.tensor_copy(dst[:, nb * 512 : (nb + 1) * 512], ps_tiles[nb][:])

        def project_compact(pool, dst, w_sb, src):
            # nb-outer: one live psum tile; used when the main pools are open
            for nb in range(4):
                ps = pool.tile([128, 512], F32, tag="psv", name="psv")
                for dk in range(DK):
                    nc.tensor.matmul(
                        ps[:],
                        w_sb[:, dk, :],
                        src[:, dk, nb * 512 : (nb + 1) * 512],
                        start=(dk == 0),
                        stop=(dk == DK - 1),
                    )
                nc.vector.tensor_copy(dst[:, nb * 512 : (nb + 1) * 512], ps[:])

        # k and q projections first so the attention loop can begin while the
        # v projection and transposes still stream through the PE
        with tc.tile_pool(name="proj_kq", bufs=2, space="PSUM") as proj_kq:
            project(proj_kq, kT, wk_sb, h_sb)
            project(proj_kq, qT, wq_sb, x_sb)

        # ---- main attention loop ----
        sc_pool = ctx.enter_context(tc.tile_pool(name="sc_ps", bufs=2, space="PSUM"))
        ctx_pool = ctx.enter_context(tc.tile_pool(name="ctx_ps", bufs=1, space="PSUM"))

        cps_all = {}

        def scores_exp(qq, kb):
            q0 = qq * QW
            k0 = kb * 128
            sc = sc_pool.tile([128, 2, QW], F32, tag="sc", name="sc")
            for h in range(2):
                f0 = h * 64
                nc.tensor.matmul(
                    sc[:, h, :],
                    kT[f0 : f0 + 64, k0 : k0 + 128],
                    qT[f0 : f0 + 64, q0 : q0 + QW],
                    start=True,
                    stop=True,
                    tile_position=(f0, 0),
                )
            at = attn_p.tile([128, 2, QW], BF16, tag="at", name="at")
            nc.scalar.activation(
                at[:], sc[:], mybir.ActivationFunctionType.Exp, scale=0.125
            )
            return at

        def ctx_mm(qq, kb, at):
            cps = cps_all[qq]
            for h, v_nat in ((0, v0), (1, v1)):
                nc.tensor.matmul(
                    cps[h][:],
                    v_nat[:, kb * 65 : (kb + 1) * 65],
                    at[:, h, :],
                    start=(kb == 0),
                    stop=(kb == KB - 1),
                )

        def drain_ctx(qq):
            # psum -> SBUF (bf16) + denominators out
            q0 = qq * QW
            cps = cps_all[qq]
            s_sb = small.tile([64, QW], F32, tag="s", name="s_sb")
            for h in range(2):
                nc.vector.tensor_copy(ctxu[h * 64 : (h + 1) * 64, q0 : q0 + QW], cps[h][0:64, :])
                nc.vector.tensor_copy(s_sb[32 * h : 32 * h + 1, :], cps[h][64:65, :])
            for h in range(2):
                nc.sync.dma_start(
                    sums_d.ap()[h : h + 1, q0 : q0 + QW], s_sb[32 * h : 32 * h + 1, :]
                )

        # quarter 0: scores/exp hoisted ahead of the v projection; its ctx
        # matmuls run after the transposes produce v_nat
        cps_all[0] = [
            ctx_pool.tile([65, QW], F32, tag=f"ctx{h}", name=f"ctx{h}") for h in range(2)
        ]
        at_q0 = [scores_exp(0, kb) for kb in range(KB)]

        with tc.tile_pool(name="proj_v", bufs=2, space="PSUM") as proj_v:
            project_compact(proj_v, vT, wv_sb, h_sb)
        with tc.tile_pool(name="tr_ps", bufs=2, space="PSUM") as tr_ps_pool:
            for kb in range(KB):
                trp = tr_ps_pool.tile([128, 128], BF16, name="trp")
                nc.tensor.transpose(trp[:], vT[:, kb * 128 : (kb + 1) * 128], ident[:])
                nc.vector.tensor_copy(v0[:, kb * 65 : kb * 65 + 64], trp[:, 0:64])
                nc.vector.tensor_copy(v1[:, kb * 65 : kb * 65 + 64], trp[:, 64:128])

        tail_pool = ctx.enter_context(tc.tile_pool(name="tail_ps", bufs=1, space="PSUM"))

        def tail_op(qq, m, last=False):
            # output projection chunk m for quarter qq: both heads as a
            # concurrent K=64 row-tiled pair, separate outputs. For the final
            # quarter the score slots are free: rotate through them too, and
            # split the drains between VectorE and the now-idle ScalarE.
            q0 = qq * QW
            if last and m % 3 != 0:
                op = sc_pool.tile([128, 2, QW], F32, tag="sc", name="op")
            else:
                op = tail_pool.tile([128, 2, QW], F32, tag="tail", name="op")
            for h in range(2):
                f0 = h * 64
                nc.tensor.matmul(
                    op[:, h, :],
                    wo_sb[f0 : f0 + 64, m * 128 : (m + 1) * 128],
                    ctxu[f0 : f0 + 64, q0 : q0 + QW],
                    start=True,
                    stop=True,
                    tile_position=(f0, 0),
                )
            o_sb = outp.tile([128, 2, QW], BF16, tag="o", name="o_sb")
            if last and m % 2 == 1:
                nc.scalar.copy(o_sb[:], op[:])
            else:
                nc.vector.tensor_copy(o_sb[:], op[:])
            # one trigger for both heads: [128, 2, QW] -> outAB[h, m-rows, q]
            nc.sync.dma_start(
                outAB_d.ap()[:, m * 128 : (m + 1) * 128, q0 : q0 + QW].rearrange(
                    "ab p q -> p ab q"
                ),
                o_sb[:],
            )

        at_q1 = [scores_exp(1, kb) for kb in range(4)]
        for kb in range(KB):
            ctx_mm(0, kb, at_q0[kb])
        drain_ctx(0)

        for qq in range(1, NQ):
            cps_all[qq] = [
                ctx_pool.tile([65, QW], F32, tag=f"ctx{h}", name=f"ctx{h}")
                for h in range(2)
            ]
            for kb in range(KB):
                if qq == 1 and kb < 4:
                    at = at_q1[kb]
                else:
                    at = scores_exp(qq, kb)
                ctx_mm(qq, kb, at)
                if 5 <= kb < 13:
                    tail_op(qq - 1, kb - 5)
            drain_ctx(qq)
        for m in range(DK):
            tail_op(NQ - 1, m, last=True)


_NC_CACHE = None


def _get_nc():
    global _NC_CACHE
    if _NC_CACHE is None:
        _NC_CACHE = build_kernel()
    return _NC_CACHE


def make_in_maps(input_embeddings, history_hidden, W_q, W_k, W_v, W_o):
    xT = np.ascontiguousarray(np.asarray(input_embeddings, dtype=np.float32).T).astype(BF16NP)
    hT = np.ascontiguousarray(np.asarray(history_hidden, dtype=np.float32).T).astype(BF16NP)
    W_q = np.asarray(W_q, dtype=np.float32)
    W_k = np.asarray(W_k, dtype=np.float32)
    W_v = np.asarray(W_v, dtype=np.float32)
    W_o = np.asarray(W_o, dtype=np.float32)
    in_maps = []
    for c in range(NCORES):
        s = slice(c * FPC, (c + 1) * FPC)
        in_maps.append(
            {
                "xT": xT,
                "hT": hT,
                "wqT": np.ascontiguousarray(W_q[s, :].T).astype(BF16NP),
                "wkT": np.ascontiguousarray(W_k[s, :].T).astype(BF16NP),
                "wvT": np.ascontiguousarray(W_v[s, :].T).astype(BF16NP),
                "woT": np.ascontiguousarray(W_o[:, s].T).astype(BF16NP),
            }
        )
    return in_maps


def kernel(input_embeddings, history_hidden, W_q, W_k, W_v, W_o, _trace=False, _trace_kwargs=None):
    nc = _get_nc()
    in_maps = make_in_maps(input_embeddings, history_hidden, W_q, W_k, W_v, W_o)
    res = run_bass_kernel_spmd(
        nc, in_maps, core_ids=list(range(NCORES)), trace=_trace, **(_trace_kwargs or {})
    )
    acc = None
    for r in res.results:
        s = r["sums"].astype(np.float64)  # [2, L]
        ab = r["outAB"].astype(np.float64)  # [2, D, L]
        part = ab[0] / s[0][None, :] + ab[1] / s[1][None, :]
        acc = part if acc is None else acc + part
    out = np.ascontiguousarray(acc.T).astype(np.float32)
    if _trace:
        kernel._last_results = res
    return out


if __name__ == "__main__":
    rng = np.random.default_rng(0)
    ins = {
        "input_embeddings": rng.standard_normal((L, D), dtype=np.float32),
        "history_hidden": rng.standard_normal((L, D), dtype=np.float32),
        "W_q": rng.standard_normal((D, D), dtype=np.float32) / 32,
        "W_k": rng.standard_normal((D, D), dtype=np.float32) / 32,
        "W_v": rng.standard_normal((D, D), dtype=np.float32) / 32,
        "W_o": rng.standard_normal((D, D), dtype=np.float32) / 32,
    }
    out = kernel(**ins)
    print("kernel output", out.shape, out.dtype, float(np.abs(out).mean()))
